# revision 49
# baseline (speedup 1.0000x reference)
"""Bass/Tile kernel for one dense transformer block (B=128,T=256,E=512,H=8,F=2048),
data-parallel over batch across 8 NeuronCores (16 batches/core).

Per-core plan (4096 tokens = 8 chunks of 2 batches / 512 tokens), software-
pipelined so the FFN of chunk c-1 executes interleaved with LN/QKV/attention
of chunk c, keeping the PE array continuously busy (HAM stays warm):

  iteration c:  [prefetch x(c+1)] LN1(c) -> QKV(c) ->
                attention(c) heads 0..15 interleaved with FFN1(c-1) blocks ->
                Wo(c)+residual -> FFN2(c-1) tb0,1 -> LN2(c) -> FFN2(c-1) tb2,3

Attention per head: S^T in one [128,384] PSUM tile ([s0 x t0..256 | s1 x t1]);
ONE exp over the whole tile (ACT, bf16 out); ONE causal-mask multiply with a
[128,384] const mask (DVE, bf16 2x); PV with [V|1] stationary giving ctx rows
0:64 + denominator row 64; reciprocal_approx_fast directly on the PSUM
denominator row; partition-broadcast of the reciprocal via a K=1 fp32r matmul
(no bf16 cast needed); ONE normalize multiply reading ctx and broadcast rec
straight from PSUM.

Engine budget per chunk (est): PE ~50us, DVE ~39us, ACT ~33us, GPSIMD ~11us.
Matmul operands bf16 (1 cyc/col); accumulation fp32 in PSUM."""

import numpy as np
from contextlib import ExitStack

import ml_dtypes
import concourse.bass as bass
import concourse.mybir as mybir
import concourse.tile as tile
from concourse import bacc
from concourse.bass import ts, ds

AF = mybir.ActivationFunctionType
ALU = mybir.AluOpType
FP32 = mybir.dt.float32
F32R = mybir.dt.float32r
BF16 = mybir.dt.bfloat16
FP8 = mybir.dt.float8e4
DR = mybir.MatmulPerfMode.DoubleRow
FP8_SCALE = 16.0  # W1/W2 pre-scaled by this; FFN2 output divided by SCALE^2

B, T, E, H = 128, 256, 512, 8
D = E // H          # 64
F = 4 * E           # 2048
NCORES = 8
BS = B // NCORES    # 16 batches per core
P = 128
EPS = 1e-5
NTOK = BS * T       # 4096 tokens per core
CT = 2 * T          # 512-token chunk = 2 batches
NCH = BS // 2       # 8 chunks per core
SM = 384            # score tile columns: [s0 x 256t | s1 x 128t]


def build(apply_gb: bool = False):
    """apply_gb=False: LN gains/biases are identity (the common case,
    verified host-side in kernel()) and are folded away entirely."""
    nc = bacc.Bacc("TRN2", target_bir_lowering=False, debug=False)

    x_d = nc.dram_tensor("x", [NTOK, E], FP32, kind="ExternalInput").ap()
    wq_d = nc.dram_tensor("wq", [E, E], BF16, kind="ExternalInput").ap()
    wk_d = nc.dram_tensor("wk", [E, E], BF16, kind="ExternalInput").ap()
    wv_d = nc.dram_tensor("wv", [E, E], BF16, kind="ExternalInput").ap()
    wo_d = nc.dram_tensor("wo", [E, E], BF16, kind="ExternalInput").ap()
    bo_d = nc.dram_tensor("bo", [E], BF16, kind="ExternalInput").ap()
    g1_d = nc.dram_tensor("g1", [E], FP32, kind="ExternalInput").ap()
    be1_d = nc.dram_tensor("be1", [E], FP32, kind="ExternalInput").ap()
    g2_d = nc.dram_tensor("g2", [E], FP32, kind="ExternalInput").ap()
    be2_d = nc.dram_tensor("be2", [E], FP32, kind="ExternalInput").ap()
    w1_d = nc.dram_tensor("w1", [E, F], FP8, kind="ExternalInput").ap()
    b1_d = nc.dram_tensor("b1", [F], FP32, kind="ExternalInput").ap()
    w2_d = nc.dram_tensor("w2", [F, E], FP8, kind="ExternalInput").ap()
    b2s_d = nc.dram_tensor("b2s", [E], BF16, kind="ExternalInput").ap()
    mask_d = nc.dram_tensor("mask384", [P, SM], BF16, kind="ExternalInput").ap()
    ident_d = nc.dram_tensor("ident", [P, P], BF16, kind="ExternalInput").ap()
    ones_d = nc.dram_tensor("ones", [1, P], BF16, kind="ExternalInput").ap()
    y_d = nc.dram_tensor("y", [NTOK, E], FP32, kind="ExternalOutput").ap()

    with tile.TileContext(nc) as tc, ExitStack() as ctx:
        # ---------------- persistent weights ----------------
        wpool = ctx.enter_context(tc.tile_pool(name="weights", bufs=1))
        wq_sb = wpool.tile([P, 4, E], BF16, name="wq_sb", tag="wq_sb")
        wk_sb = wpool.tile([P, 4, E], BF16, name="wk_sb", tag="wk_sb")
        wv_sb = wpool.tile([P, 4, E], BF16, name="wv_sb", tag="wv_sb")
        wo_sb = wpool.tile([P, 4, E], BF16, name="wo_sb", tag="wo_sb")
        w1_sb = wpool.tile([P, 4, F], FP8, name="w1_sb", tag="w1_sb")
        w2_sb = wpool.tile([P, 16, E], FP8, name="w2_sb", tag="w2_sb")
        b1_sb = wpool.tile([P, 16], FP32, name="b1_sb", tag="b1_sb")
        bo_sb = wpool.tile([1, E], BF16, name="bo_sb", tag="bo_sb")
        b2s_sb = wpool.tile([1, E], BF16, name="b2s_sb", tag="b2s_sb")
        g1_sb = wpool.tile([P, 4], FP32, name="g1_sb", tag="g1_sb")
        be1_sb = wpool.tile([P, 4], FP32, name="be1_sb", tag="be1_sb")
        g2_sb = wpool.tile([P, 4], FP32, name="g2_sb", tag="g2_sb")
        be2_sb = wpool.tile([P, 4], FP32, name="be2_sb", tag="be2_sb")
        mask_sb = wpool.tile([P, SM], BF16, name="mask_sb", tag="mask_sb")
        ident_sb = wpool.tile([P, P], BF16, name="ident_sb", tag="ident_sb")
        ones_sb = wpool.tile([1, P], BF16, name="ones_sb", tag="ones_sb")
        eps_sb = wpool.tile([P, 1], FP32, name="eps_sb", tag="eps_sb")
        # keep GPSIMD single-purpose (PartitionBroadcast only): any other op
        # class on it forces a ~7us Q7 library reload mid-kernel
        nc.vector.memset(eps_sb, EPS)

        # first x chunk before the (much larger) weight loads: LN1(0) is the
        # first consumer of any DMA result
        x0_sb = wpool.tile([P, 4, E], FP32, name="x0_sb", tag="x0_sb")
        nc.sync.dma_start(x0_sb,
                          x_d[ds(0, CT), :].rearrange("(j p) e -> p j e", p=P))
        nc.sync.dma_start(wq_sb, wq_d.rearrange("(eo ei) f -> ei eo f", ei=P))
        nc.sync.dma_start(wk_sb, wk_d.rearrange("(eo ei) f -> ei eo f", ei=P))
        nc.sync.dma_start(wv_sb, wv_d.rearrange("(eo ei) f -> ei eo f", ei=P))
        nc.sync.dma_start(wo_sb, wo_d.rearrange("(eo ei) f -> ei eo f", ei=P))
        nc.sync.dma_start(w1_sb, w1_d.rearrange("(eo ei) f -> ei eo f", ei=P))
        nc.sync.dma_start(w2_sb, w2_d.rearrange("(fo fi) e -> fi fo e", fi=P))
        nc.sync.dma_start(b1_sb, b1_d.rearrange("(fo fi) -> fi fo", fi=P))
        nc.sync.dma_start(bo_sb, bo_d[None, :])
        nc.sync.dma_start(b2s_sb, b2s_d[None, :])
        nc.sync.dma_start(g1_sb, g1_d.rearrange("(eo ei) -> ei eo", ei=P))
        nc.sync.dma_start(be1_sb, be1_d.rearrange("(eo ei) -> ei eo", ei=P))
        nc.sync.dma_start(g2_sb, g2_d.rearrange("(eo ei) -> ei eo", ei=P))
        nc.sync.dma_start(be2_sb, be2_d.rearrange("(eo ei) -> ei eo", ei=P))
        nc.sync.dma_start(mask_sb, mask_d)
        nc.sync.dma_start(ident_sb, ident_d)
        nc.sync.dma_start(ones_sb, ones_d)

        # ---------------- working pools ----------------
        sb = ctx.enter_context(tc.tile_pool(name="work", bufs=2))
        ps = ctx.enter_context(tc.tile_pool(name="psum", bufs=1, space="PSUM"))

        def ln_stats(x_ap):
            """LN stats + normalize over free dim of x_ap [128t, 512e] ->
            bf16 xhat [128t, 512e] (DVE/ACT only, no PE work)."""
            stats = sb.tile([P, 6], FP32, name="stats", tag="stats", bufs=4)
            nc.vector.bn_stats(stats, x_ap)
            mv = sb.tile([P, 2], FP32, name="mv", tag="mv", bufs=4)
            nc.vector.bn_aggr(mv, stats)
            rstd = sb.tile([P, 1], FP32, name="rstd", tag="rstd", bufs=4)
            nc.scalar.activation(rstd, mv[:, 1:2], AF.Sqrt, bias=eps_sb)
            nc.vector.reciprocal(rstd, rstd)
            xh = sb.tile([P, E], BF16, name="xh", tag="xh", bufs=4)
            nc.vector.tensor_scalar(
                out=xh, in0=x_ap, scalar1=mv[:, 0:1], scalar2=rstd,
                op0=ALU.subtract, op1=ALU.mult)
            return xh

        def ln_finish(xh, g_ap, b_ap, hT, j):
            """PE-transpose xhat into hT[:, :, ts(j,128)]; g/b applied as
            per-partition tensor_scalar ops post-transpose (or a plain copy
            when the affine is identity)."""
            ps_tr = ps.tile([P, 4, P], BF16, name="ps_tr", tag="st", bufs=2)
            for eo in range(4):
                nc.tensor.transpose(ps_tr[:, eo, :], xh[:, ts(eo, P)], ident_sb)
            if not apply_gb:
                nc.vector.tensor_copy(hT[:, :, ts(j, P)], ps_tr)
                return
            for eo in range(4):
                nc.vector.tensor_scalar(
                    out=hT[:, eo, ts(j, P)], in0=ps_tr[:, eo, :],
                    scalar1=g_ap[:, eo:eo + 1], scalar2=b_ap[:, eo:eo + 1],
                    op0=ALU.mult, op1=ALU.add)

        def qkv_proj(h1T, qT, kT, v_t):
            for m in range(4):
                for w_sb_, outT, nm in ((wq_sb, qT, "q"), (wk_sb, kT, "k")):
                    ps_p = ps.tile([P, CT], FP32, name=f"ps_{nm}", tag="big",
                                   bufs=2)
                    for ke in range(4):
                        nc.tensor.matmul(
                            ps_p, w_sb_[:, ke, ts(m, P)],
                            h1T[:, ke, :], start=(ke == 0), stop=(ke == 3))
                    nc.scalar.copy(outT[:, m, :], ps_p)
            for j in range(4):
                ps_v = ps.tile([P, E], FP32, name="ps_v", tag="big", bufs=2)
                for ke in range(4):
                    nc.tensor.matmul(
                        ps_v, h1T[:, ke, ts(j, P)], wv_sb[:, ke],
                        start=(ke == 0), stop=(ke == 3))
                nc.scalar.copy(
                    v_t[:, j, :, D:2 * D],
                    ps_v.rearrange("p (h d) -> p h d", h=H))

        def attn_head(qT, kT, v_t, ctxnT, i):
            b, h = divmod(i, H)
            t0 = b * T
            p0 = (h % 2) * 64
            hdo = h // 2
            ps_st = ps.tile([P, SM], FP32, name="ps_st", tag="st", bufs=2)
            nc.tensor.matmul(
                ps_st[:, 0:T], kT[p0:p0 + 64, hdo, ds(t0, P)],
                qT[p0:p0 + 64, hdo, ds(t0, T)], start=True, stop=True)
            nc.tensor.matmul(
                ps_st[:, T:SM], kT[p0:p0 + 64, hdo, ds(t0 + P, P)],
                qT[p0:p0 + 64, hdo, ds(t0 + P, P)], start=True, stop=True)
            eT = sb.tile([P, SM], BF16, name="eT", tag="eT", bufs=3)
            nc.scalar.activation(eT, ps_st, AF.Exp, scale=float(D) ** -0.5)
            nc.vector.tensor_mul(eT, eT, mask_sb)
            # PV stationary layout [1 | 0*63 | V]: denominator row lands at
            # partition 0 (reciprocal_approx_fast needs base-0 input), ctx at
            # partitions 64:128 (32-aligned for the normalize multiply).
            ps_pv = ps.tile([P, T], FP32, name="ps_pv", tag="pv", bufs=2)
            nc.tensor.matmul(ps_pv, v_t[:, 2 * b, h, :],
                             eT[:, 0:T], start=True, stop=False)
            nc.tensor.matmul(ps_pv[:, P:T], v_t[:, 2 * b + 1, h, :],
                             eT[:, T:SM], start=False, stop=True)
            rec = sb.tile([1, T], FP32, name="rec", tag="rec", bufs=2)
            nc.vector.reciprocal_approx_fast(rec, ps_pv[0:1, :])
            rec_bc = sb.tile([D, T], FP32, name="rec_bc", tag="rec_bc", bufs=2)
            nc.gpsimd.partition_broadcast(rec_bc, rec)
            nc.vector.tensor_mul(ctxnT[p0:p0 + 64, hdo, ds(t0, T)],
                                 ps_pv[D:2 * D, :], rec_bc)

        def ffn1_block(h2T, aT, fb):
            # fp8 DoubleRow: each matmul contracts a 256-wide e-pair block
            ps_f1 = ps.tile([P, CT], FP32, name="ps_f1", tag="big", bufs=2)
            for g in range(2):
                nc.tensor.matmul(ps_f1, w1_sb[:, 2 * g:2 * g + 2, ts(fb, P)],
                                 h2T[:, 2 * g:2 * g + 2, :],
                                 start=(g == 0), stop=(g == 1), perf_mode=DR)
            nc.scalar.activation(aT[:, fb, :], ps_f1, AF.Relu,
                                 bias=b1_sb[:, fb:fb + 1])

        def wo_block(ctxnT, x_t, x2_t, tb):
            # +bo rides the accumulation group as a K=1 ones-row matmul
            ps_o = ps.tile([P, E], FP32, name="ps_o", tag="big", bufs=2)
            for hdo in range(4):
                nc.tensor.matmul(ps_o, ctxnT[:, hdo, ts(tb, P)],
                                 wo_sb[:, hdo, :],
                                 start=(hdo == 0), stop=False)
            nc.tensor.matmul(ps_o, ones_sb, bo_sb, start=False, stop=True)
            nc.vector.tensor_add(x2_t[:, tb, :], ps_o, x_t[:, tb, :])

        def ffn2_tb(aT, x2_t, c_prev, tb):
            # fp8 DoubleRow over f-pair blocks; +b2*SCALE^2 rides the
            # accumulation group (K=1 ones-row), /SCALE^2 folded into the add
            ps_f2 = ps.tile([P, E], FP32, name="ps_f2", tag="bcf2", bufs=2)
            for g in range(8):
                nc.tensor.matmul(ps_f2, aT[:, 2 * g:2 * g + 2, ts(tb, P)],
                                 w2_sb[:, 2 * g:2 * g + 2, :],
                                 start=(g == 0), stop=False, perf_mode=DR)
            nc.tensor.matmul(ps_f2, ones_sb, b2s_sb, start=False, stop=True)
            y_t = sb.tile([P, E], FP32, name="y_t", tag="y_t", bufs=2)
            nc.vector.scalar_tensor_tensor(
                out=y_t, in0=ps_f2, scalar=1.0 / (FP8_SCALE * FP8_SCALE),
                in1=x2_t[:, tb, :], op0=ALU.mult, op1=ALU.add)
            nc.sync.dma_start(y_d[ds(c_prev * CT + tb * P, P), :], y_t)

        def load_x(c):
            x_t = sb.tile([P, 4, E], FP32, name="x_t", tag="x_t", bufs=2)
            nc.sync.dma_start(
                x_t, x_d[ds(c * CT, CT), :].rearrange("(j p) e -> p j e", p=P))
            return x_t

        x_ts = {}
        x2_ts = {}
        h2Ts = {}
        aTs = {}

        x_ts[0] = x0_sb
        for it in range(NCH + 1):
            a = it if it < NCH else None        # stage-A chunk
            p = it - 1 if it >= 1 else None     # FFN1 + FFN2 tb0/1 chunk
            p2 = it - 2 if it >= 2 else None    # FFN2 tb2/3 chunk
            if a is not None and a + 1 < NCH:
                x_ts[a + 1] = load_x(a + 1)
            if a is not None:
                if p is not None:
                    aT = sb.tile([P, 16, CT], FP8, name="aT", tag="aT",
                                 bufs=2)
                    aTs[p] = aT

                def f1(fb):
                    if p is not None:
                        ffn1_block(h2Ts[p], aTs[p], fb)

                # LN1 stats (DVE) first; PE transposes interleaved with
                # FFN1(p) blocks so the PE never waits on the stats chain.
                xhs = [ln_stats(x_ts[a][:, j, :]) for j in range(4)]
                h1T = sb.tile([P, 4, CT], BF16, name="h1T", tag="h1T", bufs=2)
                for j in range(4):
                    f1(j)
                    ln_finish(xhs[j], g1_sb, be1_sb, h1T, j)
                qT = sb.tile([P, 4, CT], BF16, name="qT", tag="qT", bufs=2)
                kT = sb.tile([P, 4, CT], BF16, name="kT", tag="kT", bufs=2)
                v_t = sb.tile([P, 4, H, P], BF16, name="v_t", tag="v_t",
                              bufs=2)
                if it < 2:
                    # set the [1|0*63|V] pad once per physical buffer; these
                    # columns are never written again, so later chunks
                    # (same two rotating buffers) reuse them.
                    nc.vector.memset(v_t[:, :, :, 1:D], 0.0)
                    nc.vector.memset(v_t[:, :, :, 0:1], 1.0)
                f1(4)
                qkv_proj(h1T, qT, kT, v_t)

                ctxnT = sb.tile([P, 4, CT], BF16, name="ctxnT", tag="ctxnT",
                                bufs=2)
                x2_t = sb.tile([P, 4, E], FP32, name="x2_t", tag="x2_t",
                               bufs=2)
                x2_ts[a] = x2_t
                xhs2 = [None] * 4

                def wo_ln2(tb):
                    # Wo + LN2 stats chain for one finished ctxnT column
                    # block (heads 0..7 fill tb0/1, heads 8..15 tb2/3)
                    wo_block(ctxnT, x_ts[a], x2_t, tb)
                    xhs2[tb] = ln_stats(x2_t[:, tb, :])

                for i in range(16):
                    attn_head(qT, kT, v_t, ctxnT, i)
                    if 5 <= i:
                        f1(i)
                    if i == 11:
                        wo_ln2(0)
                    if i == 13:
                        wo_ln2(1)
                wo_ln2(2)
                wo_ln2(3)
                h2T = sb.tile([P, 4, CT], FP8, name="h2T", tag="h2T", bufs=2)
                h2Ts[a] = h2T
                for j in range(4):
                    if p is not None:
                        ffn2_tb(aTs[p], x2_ts[p], p, j)
                    ln_finish(xhs2[j], g2_sb, be2_sb, h2T, j)
            else:
                # epilogue: full FFN of the last chunk
                aT = sb.tile([P, 16, CT], FP8, name="aT", tag="aT", bufs=2)
                aTs[p] = aT
                for fb in range(16):
                    ffn1_block(h2Ts[p], aT, fb)
                for tb in range(4):
                    ffn2_tb(aT, x2_ts[p], p, tb)

    nc.compile()
    return nc


def make_aux_inputs():
    bf = ml_dtypes.bfloat16
    triu = np.triu(np.ones((P, P), np.float32))
    mask384 = np.concatenate(
        [triu, np.ones((P, P), np.float32), triu], axis=1).astype(bf)
    ident = np.eye(P, dtype=bf)
    ones = np.ones((1, P), bf)
    return {"mask384": mask384, "ident": ident, "ones": ones}


def weight_inputs(Wq, Wk, Wv, Wo, bo, ln1_g, ln1_b, ln2_g, ln2_b, W1, b1, W2, b2):
    bf = ml_dtypes.bfloat16
    f32 = lambda a: np.ascontiguousarray(np.asarray(a), dtype=np.float32)
    tobf = lambda a: np.ascontiguousarray(np.asarray(a, dtype=np.float32)).astype(bf)
    m = {
        "wq": np.ascontiguousarray(f32(Wq).transpose(1, 0, 2).reshape(E, E)).astype(bf),
        "wk": np.ascontiguousarray(f32(Wk).transpose(1, 0, 2).reshape(E, E)).astype(bf),
        "wv": np.ascontiguousarray(f32(Wv).transpose(1, 0, 2).reshape(E, E)).astype(bf),
        "wo": tobf(Wo),
        "bo": tobf(bo), "g1": f32(ln1_g), "be1": f32(ln1_b),
        "g2": f32(ln2_g), "be2": f32(ln2_b),
        "w1": (FP8_SCALE * f32(W1)).astype(ml_dtypes.float8_e4m3),
        "b1": FP8_SCALE * f32(b1),
        "w2": (FP8_SCALE * f32(W2)).astype(ml_dtypes.float8_e4m3),
        "b2s": (FP8_SCALE * FP8_SCALE * f32(b2)).astype(ml_dtypes.bfloat16),
    }
    m.update(make_aux_inputs())
    return m

from concourse.bass_utils import run_bass_kernel_spmd

_NC_CACHE = {}


def get_compiled(apply_gb: bool = False):
    if apply_gb not in _NC_CACHE:
        _NC_CACHE[apply_gb] = build(apply_gb)
    return _NC_CACHE[apply_gb]


def run_sharded(in_maps, apply_gb: bool = False, **kwargs):
    nc = get_compiled(apply_gb)
    return run_bass_kernel_spmd(nc, in_maps, core_ids=list(range(NCORES)), **kwargs)


def make_in_maps(x, weights):
    x = np.ascontiguousarray(np.asarray(x), dtype=np.float32)
    in_maps = []
    for c in range(NCORES):
        m = dict(weights)
        m["x"] = np.ascontiguousarray(x[c * BS:(c + 1) * BS].reshape(NTOK, E))
        in_maps.append(m)
    return in_maps


def kernel(x, Wq, Wk, Wv, Wo, bo, ln1_g, ln1_b, ln2_g, ln2_b, W1, b1, W2, b2):
    apply_gb = not (
        np.all(np.asarray(ln1_g) == 1) and np.all(np.asarray(ln1_b) == 0)
        and np.all(np.asarray(ln2_g) == 1) and np.all(np.asarray(ln2_b) == 0))
    weights = weight_inputs(Wq, Wk, Wv, Wo, bo, ln1_g, ln1_b,
                            ln2_g, ln2_b, W1, b1, W2, b2)
    res = run_sharded(make_in_maps(x, weights), apply_gb=apply_gb)
    y = np.stack([res.results[c]["y"].reshape(BS, T, E)
                  for c in range(NCORES)], axis=0).reshape(B, T, E)
    return np.ascontiguousarray(y.astype(np.float32))


# revision 51
# speedup vs baseline: 1.0296x; 1.0296x over previous
"""Bass/Tile kernel for one dense transformer block (B=128,T=256,E=512,H=8,F=2048),
data-parallel over batch across 8 NeuronCores (16 batches/core).

Per-core plan (4096 tokens = 8 chunks of 2 batches / 512 tokens), software-
pipelined one chunk deep so the PE array never waits on LN/softmax chains:

  iteration c: [prefetch x(c+1)]
    LN1(c) stats (DVE) ... transposes interleaved with FFN1(c-1) blocks 0..4
    QKV(c) (bf16, N=512 streams)
    attention(c) heads 0..15, FFN1(c-1) blocks 5..15 interleaved;
      Wo(c)+LN2-stats for tb0/tb1 fire mid-loop (heads 0..7 feed them)
    Wo(c) tb2/3 + LN2(c) stats ... transposes interleaved with FFN2(c-1)

Attention per head: S^T in one [128,384] PSUM tile ([s0 x t0:256 | s1 x t1]);
ONE exp over the whole tile (ACT, bf16 out); ONE causal-mask multiply with a
[128,384] const [triu|ones|triu] mask (DVE, bf16); PV stationary padded
[1|0*63|V] so the softmax denominator lands on PSUM partition 0
(reciprocal_approx_fast requires base-0 input) and ctx on partitions 64:128;
reciprocal straight off PSUM; GPSIMD partition_broadcast (the ONLY gpsimd op
class used - anything else forces ~7us Q7 library reloads); ONE normalize
multiply (PSUM ctx x SBUF broadcast).

FFN runs in fp8e4 DoubleRow (2 K-rows/cycle): W1,W2 pre-scaled x16 on the
host to clear the e4m3 subnormal range, b1 x16, FFN2 output re-scaled by
1/256 inside the y-residual scalar_tensor_tensor. bo and b2*256 ride their
matmul accumulation groups as K=1 ones-row matmuls (no DVE bias adds).
LN affine (identity in this problem's inputs, checked host-side) is folded
away; general inputs fall back to build(apply_gb=True).

Measured: 835790ns baseline -> ~516000ns; rel err 1.24e-2 (gate 2e-2,
deterministic inputs). PE ~74% busy; DVE is the secondary bottleneck."""

import numpy as np
from contextlib import ExitStack

import ml_dtypes
import concourse.bass as bass
import concourse.mybir as mybir
import concourse.tile as tile
from concourse import bacc
from concourse.bass import ts, ds

AF = mybir.ActivationFunctionType
ALU = mybir.AluOpType
FP32 = mybir.dt.float32
F32R = mybir.dt.float32r
BF16 = mybir.dt.bfloat16
FP8 = mybir.dt.float8e4
DR = mybir.MatmulPerfMode.DoubleRow
FP8_SCALE = 16.0  # W1/W2 pre-scaled by this; FFN2 output divided by SCALE^2

B, T, E, H = 128, 256, 512, 8
D = E // H          # 64
F = 4 * E           # 2048
NCORES = 8
BS = B // NCORES    # 16 batches per core
P = 128
EPS = 1e-5
NTOK = BS * T       # 4096 tokens per core
CT = 2 * T          # 512-token chunk = 2 batches
NCH = BS // 2       # 8 chunks per core
SM = 384            # score tile columns: [s0 x 256t | s1 x 128t]


def build(apply_gb: bool = False):
    """apply_gb=False: LN gains/biases are identity (the common case,
    verified host-side in kernel()) and are folded away entirely."""
    nc = bacc.Bacc("TRN2", target_bir_lowering=False, debug=False)

    x_d = nc.dram_tensor("x", [NTOK, E], FP32, kind="ExternalInput").ap()
    wq_d = nc.dram_tensor("wq", [E, E], BF16, kind="ExternalInput").ap()
    wk_d = nc.dram_tensor("wk", [E, E], BF16, kind="ExternalInput").ap()
    wv_d = nc.dram_tensor("wv", [E, E], BF16, kind="ExternalInput").ap()
    wo_d = nc.dram_tensor("wo", [E, E], BF16, kind="ExternalInput").ap()
    bo_d = nc.dram_tensor("bo", [E], BF16, kind="ExternalInput").ap()
    g1_d = nc.dram_tensor("g1", [E], FP32, kind="ExternalInput").ap()
    be1_d = nc.dram_tensor("be1", [E], FP32, kind="ExternalInput").ap()
    g2_d = nc.dram_tensor("g2", [E], FP32, kind="ExternalInput").ap()
    be2_d = nc.dram_tensor("be2", [E], FP32, kind="ExternalInput").ap()
    w1_d = nc.dram_tensor("w1", [E, F], FP8, kind="ExternalInput").ap()
    b1_d = nc.dram_tensor("b1", [F], FP32, kind="ExternalInput").ap()
    w2_d = nc.dram_tensor("w2", [F, E], FP8, kind="ExternalInput").ap()
    b2s_d = nc.dram_tensor("b2s", [E], BF16, kind="ExternalInput").ap()
    mask_d = nc.dram_tensor("mask384", [P, SM], BF16, kind="ExternalInput").ap()
    ident_d = nc.dram_tensor("ident", [P, P], BF16, kind="ExternalInput").ap()
    ones_d = nc.dram_tensor("ones", [1, P], BF16, kind="ExternalInput").ap()
    y_d = nc.dram_tensor("y", [NTOK, E], FP32, kind="ExternalOutput").ap()

    with tile.TileContext(nc) as tc, ExitStack() as ctx:
        # ---------------- persistent weights ----------------
        wpool = ctx.enter_context(tc.tile_pool(name="weights", bufs=1))
        wq_sb = wpool.tile([P, 4, E], BF16, name="wq_sb", tag="wq_sb")
        wk_sb = wpool.tile([P, 4, E], BF16, name="wk_sb", tag="wk_sb")
        wv_sb = wpool.tile([P, 4, E], BF16, name="wv_sb", tag="wv_sb")
        wo_sb = wpool.tile([P, 4, E], BF16, name="wo_sb", tag="wo_sb")
        w1_sb = wpool.tile([P, 4, F], FP8, name="w1_sb", tag="w1_sb")
        w2_sb = wpool.tile([P, 16, E], FP8, name="w2_sb", tag="w2_sb")
        b1_sb = wpool.tile([P, 16], FP32, name="b1_sb", tag="b1_sb")
        bo_sb = wpool.tile([1, E], BF16, name="bo_sb", tag="bo_sb")
        b2s_sb = wpool.tile([1, E], BF16, name="b2s_sb", tag="b2s_sb")
        g1_sb = wpool.tile([P, 4], FP32, name="g1_sb", tag="g1_sb")
        be1_sb = wpool.tile([P, 4], FP32, name="be1_sb", tag="be1_sb")
        g2_sb = wpool.tile([P, 4], FP32, name="g2_sb", tag="g2_sb")
        be2_sb = wpool.tile([P, 4], FP32, name="be2_sb", tag="be2_sb")
        mask_sb = wpool.tile([P, SM], BF16, name="mask_sb", tag="mask_sb")
        ident_sb = wpool.tile([P, P], BF16, name="ident_sb", tag="ident_sb")
        ones_sb = wpool.tile([1, P], BF16, name="ones_sb", tag="ones_sb")
        eps_sb = wpool.tile([P, 1], FP32, name="eps_sb", tag="eps_sb")
        # keep GPSIMD single-purpose (PartitionBroadcast only): any other op
        # class on it forces a ~7us Q7 library reload mid-kernel
        nc.vector.memset(eps_sb, EPS)

        # first x chunk before the (much larger) weight loads: LN1(0) is the
        # first consumer of any DMA result
        x0_sb = wpool.tile([P, 4, E], FP32, name="x0_sb", tag="x0_sb")
        nc.sync.dma_start(x0_sb,
                          x_d[ds(0, CT), :].rearrange("(j p) e -> p j e", p=P))
        nc.sync.dma_start(wq_sb, wq_d.rearrange("(eo ei) f -> ei eo f", ei=P))
        nc.sync.dma_start(wk_sb, wk_d.rearrange("(eo ei) f -> ei eo f", ei=P))
        nc.sync.dma_start(wv_sb, wv_d.rearrange("(eo ei) f -> ei eo f", ei=P))
        nc.sync.dma_start(wo_sb, wo_d.rearrange("(eo ei) f -> ei eo f", ei=P))
        nc.sync.dma_start(w1_sb, w1_d.rearrange("(eo ei) f -> ei eo f", ei=P))
        nc.sync.dma_start(w2_sb, w2_d.rearrange("(fo fi) e -> fi fo e", fi=P))
        nc.sync.dma_start(b1_sb, b1_d.rearrange("(fo fi) -> fi fo", fi=P))
        nc.sync.dma_start(bo_sb, bo_d[None, :])
        nc.sync.dma_start(b2s_sb, b2s_d[None, :])
        nc.sync.dma_start(g1_sb, g1_d.rearrange("(eo ei) -> ei eo", ei=P))
        nc.sync.dma_start(be1_sb, be1_d.rearrange("(eo ei) -> ei eo", ei=P))
        nc.sync.dma_start(g2_sb, g2_d.rearrange("(eo ei) -> ei eo", ei=P))
        nc.sync.dma_start(be2_sb, be2_d.rearrange("(eo ei) -> ei eo", ei=P))
        nc.sync.dma_start(mask_sb, mask_d)
        nc.sync.dma_start(ident_sb, ident_d)
        nc.sync.dma_start(ones_sb, ones_d)

        # ---------------- working pools ----------------
        sb = ctx.enter_context(tc.tile_pool(name="work", bufs=2))
        ps = ctx.enter_context(tc.tile_pool(name="psum", bufs=1, space="PSUM"))

        def ln_stats(x_ap):
            """LN stats + normalize over free dim of x_ap [128t, 512e] ->
            bf16 xhat [128t, 512e] (DVE/ACT only, no PE work)."""
            stats = sb.tile([P, 6], FP32, name="stats", tag="stats", bufs=4)
            nc.vector.bn_stats(stats, x_ap)
            mv = sb.tile([P, 2], FP32, name="mv", tag="mv", bufs=4)
            nc.vector.bn_aggr(mv, stats)
            rstd = sb.tile([P, 1], FP32, name="rstd", tag="rstd", bufs=4)
            nc.scalar.activation(rstd, mv[:, 1:2], AF.Sqrt, bias=eps_sb)
            nc.vector.reciprocal(rstd, rstd)
            xh = sb.tile([P, E], BF16, name="xh", tag="xh", bufs=4)
            nc.vector.tensor_scalar(
                out=xh, in0=x_ap, scalar1=mv[:, 0:1], scalar2=rstd,
                op0=ALU.subtract, op1=ALU.mult)
            return xh

        def ln_finish(xh, g_ap, b_ap, hT, j):
            """PE-transpose xhat into hT[:, :, ts(j,128)]; g/b applied as
            per-partition tensor_scalar ops post-transpose (or a plain copy
            when the affine is identity)."""
            ps_tr = ps.tile([P, 4, P], BF16, name="ps_tr", tag="st", bufs=2)
            for eo in range(4):
                nc.tensor.transpose(ps_tr[:, eo, :], xh[:, ts(eo, P)], ident_sb)
            if not apply_gb:
                nc.vector.tensor_copy(hT[:, :, ts(j, P)], ps_tr)
                return
            for eo in range(4):
                nc.vector.tensor_scalar(
                    out=hT[:, eo, ts(j, P)], in0=ps_tr[:, eo, :],
                    scalar1=g_ap[:, eo:eo + 1], scalar2=b_ap[:, eo:eo + 1],
                    op0=ALU.mult, op1=ALU.add)

        def qkv_proj(h1T, qT, kT, v_t):
            for m in range(4):
                for w_sb_, outT, nm in ((wq_sb, qT, "q"), (wk_sb, kT, "k")):
                    ps_p = ps.tile([P, CT], FP32, name=f"ps_{nm}", tag="big",
                                   bufs=2)
                    for ke in range(4):
                        nc.tensor.matmul(
                            ps_p, w_sb_[:, ke, ts(m, P)],
                            h1T[:, ke, :], start=(ke == 0), stop=(ke == 3))
                    nc.scalar.copy(outT[:, m, :], ps_p)
            for j in range(4):
                ps_v = ps.tile([P, E], FP32, name="ps_v", tag="big", bufs=2)
                for ke in range(4):
                    nc.tensor.matmul(
                        ps_v, h1T[:, ke, ts(j, P)], wv_sb[:, ke],
                        start=(ke == 0), stop=(ke == 3))
                nc.vector.tensor_copy(
                    v_t[:, j, :, D:2 * D],
                    ps_v.rearrange("p (h d) -> p h d", h=H))

        def attn_head(qT, kT, v_t, ctxnT, i):
            b, h = divmod(i, H)
            t0 = b * T
            p0 = (h % 2) * 64
            hdo = h // 2
            ps_st = ps.tile([P, SM], FP32, name="ps_st", tag="st", bufs=2)
            nc.tensor.matmul(
                ps_st[:, 0:T], kT[p0:p0 + 64, hdo, ds(t0, P)],
                qT[p0:p0 + 64, hdo, ds(t0, T)], start=True, stop=True)
            nc.tensor.matmul(
                ps_st[:, T:SM], kT[p0:p0 + 64, hdo, ds(t0 + P, P)],
                qT[p0:p0 + 64, hdo, ds(t0 + P, P)], start=True, stop=True)
            eT = sb.tile([P, SM], BF16, name="eT", tag="eT", bufs=3)
            nc.scalar.activation(eT, ps_st, AF.Exp, scale=float(D) ** -0.5)
            nc.vector.tensor_mul(eT, eT, mask_sb)
            # PV stationary layout [1 | 0*63 | V]: denominator row lands at
            # partition 0 (reciprocal_approx_fast needs base-0 input), ctx at
            # partitions 64:128 (32-aligned for the normalize multiply).
            ps_pv = ps.tile([P, T], FP32, name="ps_pv", tag="pv", bufs=2)
            nc.tensor.matmul(ps_pv, v_t[:, 2 * b, h, :],
                             eT[:, 0:T], start=True, stop=False)
            nc.tensor.matmul(ps_pv[:, P:T], v_t[:, 2 * b + 1, h, :],
                             eT[:, T:SM], start=False, stop=True)
            rec = sb.tile([1, T], FP32, name="rec", tag="rec", bufs=2)
            nc.vector.reciprocal_approx_fast(rec, ps_pv[0:1, :])
            rec_bc = sb.tile([D, T], FP32, name="rec_bc", tag="rec_bc", bufs=2)
            nc.gpsimd.partition_broadcast(rec_bc, rec)
            nc.vector.tensor_mul(ctxnT[p0:p0 + 64, hdo, ds(t0, T)],
                                 ps_pv[D:2 * D, :], rec_bc)

        def ffn1_block(h2T, aT, fb):
            # fp8 DoubleRow: each matmul contracts a 256-wide e-pair block
            ps_f1 = ps.tile([P, CT], FP32, name="ps_f1", tag="big", bufs=2)
            for g in range(2):
                nc.tensor.matmul(ps_f1, w1_sb[:, 2 * g:2 * g + 2, ts(fb, P)],
                                 h2T[:, 2 * g:2 * g + 2, :],
                                 start=(g == 0), stop=(g == 1), perf_mode=DR)
            nc.scalar.activation(aT[:, fb, :], ps_f1, AF.Relu,
                                 bias=b1_sb[:, fb:fb + 1])

        def wo_block(ctxnT, x_t, x2_t, tb):
            # +bo rides the accumulation group as a K=1 ones-row matmul
            ps_o = ps.tile([P, E], FP32, name="ps_o", tag="big", bufs=2)
            for hdo in range(4):
                nc.tensor.matmul(ps_o, ctxnT[:, hdo, ts(tb, P)],
                                 wo_sb[:, hdo, :],
                                 start=(hdo == 0), stop=False)
            nc.tensor.matmul(ps_o, ones_sb, bo_sb, start=False, stop=True)
            nc.vector.tensor_add(x2_t[:, tb, :], ps_o, x_t[:, tb, :])

        def ffn2_tb(aT, x2_t, c_prev, tb):
            # fp8 DoubleRow over f-pair blocks; +b2*SCALE^2 rides the
            # accumulation group (K=1 ones-row), /SCALE^2 folded into the add
            ps_f2 = ps.tile([P, E], FP32, name="ps_f2", tag="bcf2", bufs=2)
            for g in range(8):
                nc.tensor.matmul(ps_f2, aT[:, 2 * g:2 * g + 2, ts(tb, P)],
                                 w2_sb[:, 2 * g:2 * g + 2, :],
                                 start=(g == 0), stop=False, perf_mode=DR)
            nc.tensor.matmul(ps_f2, ones_sb, b2s_sb, start=False, stop=True)
            y_t = sb.tile([P, E], FP32, name="y_t", tag="y_t", bufs=2)
            nc.vector.scalar_tensor_tensor(
                out=y_t, in0=ps_f2, scalar=1.0 / (FP8_SCALE * FP8_SCALE),
                in1=x2_t[:, tb, :], op0=ALU.mult, op1=ALU.add)
            nc.sync.dma_start(y_d[ds(c_prev * CT + tb * P, P), :], y_t)

        def load_x(c):
            x_t = sb.tile([P, 4, E], FP32, name="x_t", tag="x_t", bufs=2)
            nc.sync.dma_start(
                x_t, x_d[ds(c * CT, CT), :].rearrange("(j p) e -> p j e", p=P))
            return x_t

        x_ts = {}
        x2_ts = {}
        h2Ts = {}
        aTs = {}

        x_ts[0] = x0_sb
        for it in range(NCH + 1):
            a = it if it < NCH else None        # stage-A chunk
            p = it - 1 if it >= 1 else None     # FFN1 + FFN2 tb0/1 chunk
            p2 = it - 2 if it >= 2 else None    # FFN2 tb2/3 chunk
            if a is not None and a + 1 < NCH:
                x_ts[a + 1] = load_x(a + 1)
            if a is not None:
                if p is not None:
                    aT = sb.tile([P, 16, CT], FP8, name="aT", tag="aT",
                                 bufs=2)
                    aTs[p] = aT

                def f1(fb):
                    if p is not None:
                        ffn1_block(h2Ts[p], aTs[p], fb)

                # LN1 stats (DVE) first; PE transposes interleaved with
                # FFN1(p) blocks so the PE never waits on the stats chain.
                xhs = [ln_stats(x_ts[a][:, j, :]) for j in range(4)]
                h1T = sb.tile([P, 4, CT], BF16, name="h1T", tag="h1T", bufs=2)
                for j in range(4):
                    f1(j)
                    ln_finish(xhs[j], g1_sb, be1_sb, h1T, j)
                qT = sb.tile([P, 4, CT], BF16, name="qT", tag="qT", bufs=2)
                kT = sb.tile([P, 4, CT], BF16, name="kT", tag="kT", bufs=2)
                v_t = sb.tile([P, 4, H, P], BF16, name="v_t", tag="v_t",
                              bufs=2)
                if it < 2:
                    # set the [1|0*63|V] pad once per physical buffer; these
                    # columns are never written again, so later chunks
                    # (same two rotating buffers) reuse them.
                    nc.vector.memset(v_t[:, :, :, 1:D], 0.0)
                    nc.vector.memset(v_t[:, :, :, 0:1], 1.0)
                f1(4)
                qkv_proj(h1T, qT, kT, v_t)

                ctxnT = sb.tile([P, 4, CT], BF16, name="ctxnT", tag="ctxnT",
                                bufs=2)
                x2_t = sb.tile([P, 4, E], FP32, name="x2_t", tag="x2_t",
                               bufs=2)
                x2_ts[a] = x2_t
                xhs2 = [None] * 4

                def wo_ln2(tb):
                    # Wo + LN2 stats chain for one finished ctxnT column
                    # block (heads 0..7 fill tb0/1, heads 8..15 tb2/3)
                    wo_block(ctxnT, x_ts[a], x2_t, tb)
                    xhs2[tb] = ln_stats(x2_t[:, tb, :])

                for i in range(16):
                    attn_head(qT, kT, v_t, ctxnT, i)
                    if 5 <= i:
                        f1(i)
                    if i == 11:
                        wo_ln2(0)
                    if i == 13:
                        wo_ln2(1)
                wo_ln2(2)
                wo_ln2(3)
                h2T = sb.tile([P, 4, CT], FP8, name="h2T", tag="h2T", bufs=2)
                h2Ts[a] = h2T
                for j in range(4):
                    if p is not None:
                        ffn2_tb(aTs[p], x2_ts[p], p, j)
                    ln_finish(xhs2[j], g2_sb, be2_sb, h2T, j)
            else:
                # epilogue: full FFN of the last chunk
                aT = sb.tile([P, 16, CT], FP8, name="aT", tag="aT", bufs=2)
                aTs[p] = aT
                for fb in range(16):
                    ffn1_block(h2Ts[p], aT, fb)
                for tb in range(4):
                    ffn2_tb(aT, x2_ts[p], p, tb)

    nc.compile()
    return nc


def make_aux_inputs():
    bf = ml_dtypes.bfloat16
    triu = np.triu(np.ones((P, P), np.float32))
    mask384 = np.concatenate(
        [triu, np.ones((P, P), np.float32), triu], axis=1).astype(bf)
    ident = np.eye(P, dtype=bf)
    ones = np.ones((1, P), bf)
    return {"mask384": mask384, "ident": ident, "ones": ones}


def weight_inputs(Wq, Wk, Wv, Wo, bo, ln1_g, ln1_b, ln2_g, ln2_b, W1, b1, W2, b2):
    bf = ml_dtypes.bfloat16
    f32 = lambda a: np.ascontiguousarray(np.asarray(a), dtype=np.float32)
    tobf = lambda a: np.ascontiguousarray(np.asarray(a, dtype=np.float32)).astype(bf)
    m = {
        "wq": np.ascontiguousarray(f32(Wq).transpose(1, 0, 2).reshape(E, E)).astype(bf),
        "wk": np.ascontiguousarray(f32(Wk).transpose(1, 0, 2).reshape(E, E)).astype(bf),
        "wv": np.ascontiguousarray(f32(Wv).transpose(1, 0, 2).reshape(E, E)).astype(bf),
        "wo": tobf(Wo),
        "bo": tobf(bo), "g1": f32(ln1_g), "be1": f32(ln1_b),
        "g2": f32(ln2_g), "be2": f32(ln2_b),
        "w1": (FP8_SCALE * f32(W1)).astype(ml_dtypes.float8_e4m3),
        "b1": FP8_SCALE * f32(b1),
        "w2": (FP8_SCALE * f32(W2)).astype(ml_dtypes.float8_e4m3),
        "b2s": (FP8_SCALE * FP8_SCALE * f32(b2)).astype(ml_dtypes.bfloat16),
    }
    m.update(make_aux_inputs())
    return m

from concourse.bass_utils import run_bass_kernel_spmd

_NC_CACHE = {}


def get_compiled(apply_gb: bool = False):
    if apply_gb not in _NC_CACHE:
        _NC_CACHE[apply_gb] = build(apply_gb)
    return _NC_CACHE[apply_gb]


def run_sharded(in_maps, apply_gb: bool = False, **kwargs):
    nc = get_compiled(apply_gb)
    return run_bass_kernel_spmd(nc, in_maps, core_ids=list(range(NCORES)), **kwargs)


def make_in_maps(x, weights):
    x = np.ascontiguousarray(np.asarray(x), dtype=np.float32)
    in_maps = []
    for c in range(NCORES):
        m = dict(weights)
        m["x"] = np.ascontiguousarray(x[c * BS:(c + 1) * BS].reshape(NTOK, E))
        in_maps.append(m)
    return in_maps


def kernel(x, Wq, Wk, Wv, Wo, bo, ln1_g, ln1_b, ln2_g, ln2_b, W1, b1, W2, b2):
    apply_gb = not (
        np.all(np.asarray(ln1_g) == 1) and np.all(np.asarray(ln1_b) == 0)
        and np.all(np.asarray(ln2_g) == 1) and np.all(np.asarray(ln2_b) == 0))
    weights = weight_inputs(Wq, Wk, Wv, Wo, bo, ln1_g, ln1_b,
                            ln2_g, ln2_b, W1, b1, W2, b2)
    res = run_sharded(make_in_maps(x, weights), apply_gb=apply_gb)
    y = np.stack([res.results[c]["y"].reshape(BS, T, E)
                  for c in range(NCORES)], axis=0).reshape(B, T, E)
    return np.ascontiguousarray(y.astype(np.float32))
